# revision 20
# baseline (speedup 1.0000x reference)
"""GQA causal attention block (sparse_attention) on 8 Trainium2 NeuronCores.

Tensor-parallel over heads: core i computes q-heads 4i..4i+3 and kv-head i
(N_KV == n_cores, so each core owns exactly one kv head), plus the matching
row-slice of the o_proj; the 8 partial o_proj outputs are summed on the host.

Layout choice: everything that feeds the PE keeps the contraction dim on
partitions. Projections produce qT/kT/vT [d, s] directly (stationary = weight
chunk, moving = xT), attention scores are computed transposed [t, s]
(stationary = kT slice, moving = qT), PV consumes v [t, d] (stationary) times
exp-scores [t, s] (moving), and o_proj consumes outT [d, s] as stationary.

Softmax denominators: exp tiles are accumulated elementwise on the Pool
engine (f32), and a single ones-matmul per (head, block) does the final
partition reduction — the PE no longer streams every exp tile twice.
Causal masking on diagonal tiles is a 0/1 upper-triangular multiply on the
exp tile (DVE) instead of a mask-add matmul. Projection/weight DMAs move in
4-chunk groups from partition-major host layouts to cut descriptor-issue
serialization. v tiles are transposed by the DMA XBAR, not the PE.
Reciprocals/rsqrts are exp(-ln(x)) on ACT (one table set: exp/ln/square).
"""

import sys

sys.path.insert(0, "/opt/trn_rl_repo")

import numpy as np
import ml_dtypes

import concourse.bass as bass
import concourse.mybir as mybir
from concourse import tile
from concourse.vector_clock import ScopedClock, VectorClock
from concourse.bass_utils import run_bass_kernel_spmd

F32 = mybir.dt.float32
BF16 = mybir.dt.bfloat16
AF = mybir.ActivationFunctionType
OP = mybir.AluOpType

S = 2048
HID = 4096
N_HEADS = 32
N_KV = 8
D = 128
NCORES = 8
QH = N_HEADS // NCORES          # q heads per core
EPS = 1e-6
SM_SCALE = float(D) ** -0.5
NJ = S // 512                   # 512-wide s blocks
NHC = HID // 128                # 128-deep contraction chunks
NT = S // 128                   # 128-tall t tiles
GC = 4                          # contraction chunks per DMA group
NG = NHC // GC                  # DMA groups per s block


class TileContextFixed(tile.TileContext):
    """TileContext whose tail drain emits one sem-wait per Drain instruction.

    The pinned walrus (CoreV3GenImpl setupSyncWait) rejects instructions that
    carry more than one sync-wait command; stock TileContext attaches the
    whole global clock to a single Drain.
    """

    def _drain_and_barrier(self, tick_clock, wait_clock):
        gc = tick_clock.global_clock
        nprocs = len(gc)
        emitted = False
        for proc in range(nprocs):
            tick = gc[proc]
            if tick <= 0:
                continue
            vec = [0] * nprocs
            vec[proc] = tick
            d = self.nc.sync.drain()
            wait_clock.add_sem_waits(d.ins, ScopedClock({None: VectorClock(vec)}))
            emitted = True
        if not emitted:
            self.nc.sync.drain()

        self.nc.all_engine_barrier()
        assert self.sems is not None
        popped = self.nc._tile_sem_poison_stack.pop()
        assert popped is self._sem_poison
        self.nc.clear_and_free_semaphores(list(self.sems.allocated().values()))
        self.nc.all_engine_barrier()


def _split_multi_waits(nc):
    """Hoist all-but-one sem wait of any instruction onto preceding NOPs.

    The pinned walrus rejects instructions with more than one sync-wait
    command; engine streams execute in order, so a same-engine NOP carrying
    the extra waits right before the instruction is equivalent.
    """
    n = 0
    for f in nc.m.functions:
        for bb in f.blocks:
            rebuilt = []
            changed = False
            for inst in bb.instructions:
                si = inst.sync_info
                if si is not None and len(si.on_wait) > 1:
                    waits = list(si.on_wait)
                    for w in waits[:-1]:
                        n += 1
                        nop = mybir.InstNoOp(
                            name=f"I-waitsplit-{n}",
                            engine=inst.engine,
                            sync_info=mybir.SyncInfo(on_wait=[w], on_update=[]),
                            bass_nofuse=True,
                        )
                        nc.register_instruction(nop)
                        rebuilt.append(nop)
                    inst.sync_info = mybir.SyncInfo(
                        on_wait=[waits[-1]], on_update=list(si.on_update)
                    )
                    changed = True
                rebuilt.append(inst)
            if changed:
                bb.instructions = rebuilt


PHASE_MARKS = []


_SCOPE = [None, None]


def _mark(nc, label):
    if _SCOPE[0] is not None:
        nc.leave_named_scope(_SCOPE[0], _SCOPE[1], False)
        _SCOPE[0] = None
    if label is not None:
        sid, _ = nc.enter_named_scope(label, False)
        _SCOPE[0], _SCOPE[1] = label, sid


def build_program():
    nc = bass.Bass()

    # partition-major host layouts: one DMA covers several contraction chunks
    # xt[j, d, hc, e] = x[512j+e, 128hc+d]
    xt = nc.dram_tensor("xt", [NJ, D, NHC, 512], BF16, kind="ExternalInput")
    # wqkv[d, hc, f] = W[128hc+d, f] with f packed [4*D q | D k | D v]
    wqkv = nc.dram_tensor("wqkv", [D, NHC, (QH + 2) * D], BF16,
                          kind="ExternalInput")
    wo = nc.dram_tensor("wo", [QH * D, HID], BF16, kind="ExternalInput")
    # packed rope tables: [:, 0, :] = cos*w; [:, 1, :] = half-swapped rotate
    # table swS with swS[d] = sign(pair(d))*sin[pair(d)]*w[d], so that
    # rot-half multiplies read both SBUF operands at the same base partition
    tabq = nc.dram_tensor("tabq", [D, 2, S], BF16, kind="ExternalInput")
    tabk = nc.dram_tensor("tabk", [D, 2, S], BF16, kind="ExternalInput")
    tri01b = nc.dram_tensor("tri01b", [D, D], BF16, kind="ExternalInput")
    out = nc.dram_tensor("out", [S, HID], BF16, kind="ExternalOutput")

    with TileContextFixed(nc) as tc:
        with (
            tc.tile_pool(name="const", bufs=1) as constp,
            tc.tile_pool(name="persist", bufs=1) as persist,
            tc.tile_pool(name="wstream", bufs=4) as wstream,
            tc.tile_pool(name="xstream", bufs=4) as xstream,
            tc.tile_pool(name="tmp", bufs=2) as tmp,
            tc.tile_pool(name="tabstream", bufs=4) as tabstream,
            tc.tile_pool(name="expp", bufs=8) as expp,
            tc.tile_pool(name="accp", bufs=3) as accp,
            tc.tile_pool(name="outsb", bufs=2) as outsb,
            tc.tile_pool(name="ps", bufs=8, space="PSUM") as ps,
        ):
            ones = constp.tile([D, D], BF16, tag="ones")
            nc.vector.memset(ones[:], 1.0)

            def issue_group(j, g, split=False):
                xg = xstream.tile([D, GC, 512], BF16, tag="xg", name="xg")
                wg = wstream.tile([D, GC, (QH + 2) * D], BF16, tag="wg",
                                  name="wg")
                weng = nc.gpsimd if split else nc.sync
                if split:
                    hh = GC // 2
                    for s in range(2):
                        cs = slice(GC * g + hh * s, GC * g + hh * (s + 1))
                        ts = slice(hh * s, hh * (s + 1))
                        nc.sync.dma_start(xg[:, ts, :], xt[j, :, cs, :])
                        weng.dma_start(wg[:, ts, :], wqkv[:, cs, :])
                else:
                    nc.sync.dma_start(xg[:], xt[j, :, GC * g:GC * (g + 1), :])
                    weng.dma_start(wg[:], wqkv[:, GC * g:GC * (g + 1), :])
                return xg, wg

            def issue_tabs(j):
                js = slice(512 * j, 512 * (j + 1))
                tabs = []
                for nm, tdram in (("tq", tabq), ("tk", tabk)):
                    tab = tabstream.tile([D, 2, 512], BF16, tag=nm, name=nm)
                    nc.sync.dma_start(tab[:], tdram[:, :, js])
                    tabs.append(tab)
                return tabs

            # prefetch the whole first block's feed before the warmup so the
            # rings fill during the cold preamble
            pre0 = [issue_group(0, g, split=True) for g in range(NG)]
            tabs0 = issue_tabs(0)

            epsb = constp.tile([D, 1], F32, tag="epsb")
            nc.vector.memset(epsb[:], EPS)
            tri01 = constp.tile([D, D], BF16, tag="tri01")
            nc.gpsimd.dma_start(tri01[:], tri01b[:])

            wosb = persist.tile([D, QH, HID], BF16, tag="wosb")

            # warm-up matmuls: keep the PE busy during the cold DMA ramp so
            # the HAM clock gate opens before the first projection matmuls
            pwarm = ps.tile([D, 512], F32, tag="ps", name="pwarm")
            for _w in range(32):
                nc.tensor.matmul(pwarm[:, 0:D], ones[:], ones[:],
                                 start=(_w == 0), stop=(_w == 31))

            qhat = [persist.tile([D, S], BF16, tag=f"qhat{h}", name=f"qhat{h}")
                    for h in range(QH)]
            khat = persist.tile([D, S], BF16, tag="khat")
            vsb = persist.tile([D, NT, D], BF16, tag="vsb")
            outt = [persist.tile([D, S], BF16, tag=f"outt{h}", name=f"outt{h}")
                    for h in range(QH)]

            def emit_proj(j, prefetched=None, tabs=None):
                """Projections for s block j + immediate PSUM evictions.

                Returns the evicted raw projections (SBUF) for the rope stage.
                """
                _mark(nc, f"proj{j}")
                pq = [ps.tile([D, 512], F32, tag="ps", name=f"pq{_h}")
                      for _h in range(QH)]
                pk = ps.tile([D, 512], F32, tag="ps", name="pk")
                pv = ps.tile([D, 512], F32, tag="ps", name="pv")
                for g in range(NG):
                    if prefetched is not None:
                        xg, wg = prefetched[g]
                    else:
                        xg, wg = issue_group(j, g)
                    if g == 1 and tabs is None:
                        tabs = issue_tabs(j)
                    for c in range(GC):
                        hc = GC * g + c
                        st = dict(start=(hc == 0), stop=(hc == NHC - 1))
                        for h in range(QH):
                            nc.tensor.matmul(pq[h][:],
                                             wg[:, c, 128 * h:128 * (h + 1)],
                                             xg[:, c, :], **st)
                        nc.tensor.matmul(pk[:], wg[:, c, QH * D:(QH + 1) * D],
                                         xg[:, c, :], **st)
                        nc.tensor.matmul(pv[:], wg[:, c, (QH + 1) * D:],
                                         xg[:, c, :], **st)

                return pq, pk, pv, tabs

            def emit_evict(psrc, h, sq_on_act=False):
                qraw = tmp.tile([D, 512], BF16, tag="qraw", bufs=6,
                                name="qraw")
                nc.vector.tensor_copy(qraw[:], psrc[:])
                sq = tmp.tile([D, 512], BF16, tag="sq", bufs=6, name="sq")
                if sq_on_act:
                    nc.scalar.activation(sq[:], psrc[:], AF.Square,
                                         bias=0.0, scale=1.0)
                else:
                    nc.vector.tensor_tensor(sq[:], qraw[:], qraw[:], OP.mult)
                return (h, qraw, sq)

            def emit_rope(j, qraws, vt, tabs):
                """RMS-norm + rope (k first) + v transpose for s block j."""
                _mark(nc, f"rope{j}")
                js = slice(512 * j, 512 * (j + 1))
                tabqt, tabkt = tabs
                for h, qraw, sq in qraws:
                    if h < QH:
                        dstt, tab = qhat[h], tabqt
                    else:
                        dstt, tab = khat, tabkt
                    pss = ps.tile([D, 512], F32, tag="ps", name="pss")
                    nc.tensor.matmul(pss[:], ones[:], sq[:], start=True,
                                     stop=True)
                    # r = rsqrt(mean + eps) = exp(-0.5 * ln(sumsq/128 + eps))
                    rbc = tmp.tile([D, 512], BF16, tag="rbc", name="rbc")
                    nc.scalar.activation(rbc[:], pss[:], AF.Ln,
                                         bias=epsb[:], scale=1.0 / D)
                    nc.scalar.activation(rbc[:], rbc[:], AF.Exp, bias=0.0,
                                         scale=-0.5)
                    t1 = tmp.tile([D, 512], BF16, tag="t1", name="t1")
                    nc.vector.tensor_tensor(t1[:], qraw[:], tab[:, 0, :],
                                            OP.mult)
                    t2 = tmp.tile([D, 512], BF16, tag="t2", name="t2")
                    nc.vector.tensor_tensor(t2[0:64, :], qraw[64:128, :],
                                            tab[64:128, 1, :], OP.mult)
                    nc.vector.tensor_tensor(t2[64:128, :], qraw[0:64, :],
                                            tab[0:64, 1, :], OP.mult)
                    nc.vector.tensor_tensor(t1[:], t1[:], t2[:], OP.add)
                    nc.vector.tensor_tensor(dstt[:, js], t1[:], rbc[:],
                                            OP.mult)

                for c in range(4):
                    nc.sync.dma_start_transpose(
                        vsb[:, 4 * j + c, :], vt[:, 128 * c:128 * (c + 1)])

            def emit_attn_head(j, h, clo=0, chi=512):
                """Attention for s block j, head h, q cols [clo, chi)."""
                if True:
                    _mark(nc, f"attn{j}.h{h}")
                    po = ps.tile([D, 512], F32, tag="ps", name="po")
                    acc = accp.tile([D, 512], BF16, tag="acc", name="acc")
                    tiles = []
                    for tt in range(4 * j + 4):
                        wlo = 128 * (tt - 4 * j)   # cols < wlo fully masked
                        if wlo >= chi:
                            continue
                        c0 = max(clo, wlo, 0)
                        tiles.append((tt, c0, wlo))
                    pending = []
                    first = True
                    for idx, (tt, c0, wlo) in enumerate(tiles):
                        cs = slice(c0, chi)
                        psc = ps.tile([D, 512], F32, tag="ps", name="psc")
                        nc.tensor.matmul(psc[:, cs],
                                         khat[:, 128 * tt:128 * (tt + 1)],
                                         qhat[h][:, 512 * j + c0:
                                                 512 * j + chi],
                                         start=True, stop=True)
                        ex = expp.tile([D, 512], BF16, tag="ex", name="ex")
                        nc.scalar.activation(ex[:, cs], psc[:, cs], AF.Exp,
                                             bias=0.0, scale=SM_SCALE)
                        if clo <= wlo:   # triangular boundary chunk
                            nc.gpsimd.tensor_tensor(
                                ex[:, wlo:wlo + 128], ex[:, wlo:wlo + 128],
                                tri01[:], OP.mult)
                        # softmax denominator: elementwise-accumulate exp
                        # tiles on DVE in bf16 (all-SBUF 2-byte ops hit the
                        # 4x DVE mode; partition sum comes later from one
                        # ones-matmul)
                        if first:
                            nc.vector.tensor_copy(acc[:, cs], ex[:, cs])
                            first = False
                        else:
                            nc.vector.tensor_tensor(acc[:, cs], acc[:, cs],
                                                    ex[:, cs], OP.add)
                        pending.append((idx, ex, cs))
                        # keep the PE a couple of score tiles ahead of the
                        # exp chain
                        if len(pending) > 2:
                            pidx, pex, pcs = pending.pop(0)
                            nc.tensor.matmul(po[:, pcs],
                                             vsb[:, tiles[pidx][0], :],
                                             pex[:, pcs],
                                             start=(pidx == 0),
                                             stop=(pidx == len(tiles) - 1))
                    for pidx, pex, pcs in pending:
                        nc.tensor.matmul(po[:, pcs], vsb[:, tiles[pidx][0], :],
                                         pex[:, pcs], start=(pidx == 0),
                                         stop=(pidx == len(tiles) - 1))
                    pd = ps.tile([D, 512], F32, tag="ps", name="pd")
                    nc.tensor.matmul(pd[:, clo:chi], ones[:],
                                     acc[:, clo:chi], start=True, stop=True)
                    # only the PSUM-freeing Ln runs inline; the Exp and the
                    # outt normalize are deferred past the head loop so the
                    # ACT queue streams exps uninterrupted during attention
                    rd = tmp.tile([D, 512], F32, tag="rd", bufs=4, name="rd")
                    nc.scalar.activation(rd[:, clo:chi], pd[:, clo:chi],
                                         AF.Ln, bias=0.0, scale=1.0)
                    return po, rd

            def finish_attn_head(j, h, po, rd, clo=0, chi=512):
                rdb = tmp.tile([D, 512], BF16, tag="rdb", bufs=4, name="rdb")
                nc.scalar.activation(rdb[:, clo:chi], rd[:, clo:chi],
                                     AF.Exp, bias=0.0, scale=-1.0)
                nc.vector.tensor_tensor(
                    outt[h][:, 512 * j + clo:512 * j + chi],
                    po[:, clo:chi], rdb[:, clo:chi], OP.mult)

            def emit_oproj(j, stts):
                """o_proj rows for s tiles `stts` (within block j)."""
                _mark(nc, f"oproj{j}")
                for stt in stts:
                    ss = slice(128 * stt, 128 * (stt + 1))
                    for half in range(2):
                        pb = [ps.tile([D, 512], F32, tag="ps", name=f"pb{_b}")
                              for _b in range(4)]
                        for h in range(QH):
                            for b in range(4):
                                col = 2048 * half + 512 * b
                                nc.tensor.matmul(pb[b][:], outt[h][:, ss],
                                                 wosb[:, h, col:col + 512],
                                                 start=(h == 0),
                                                 stop=(h == QH - 1))
                        osb = outsb.tile([D, 2048], BF16, tag="osb",
                                         name="osb")
                        for b in range(4):
                            if b % 2 == 0:
                                nc.scalar.copy(osb[:, 512 * b:512 * (b + 1)],
                                               pb[b][:])
                            else:
                                nc.vector.tensor_copy(
                                    osb[:, 512 * b:512 * (b + 1)], pb[b][:])
                        nc.gpsimd.dma_start(
                            out[ss, 2048 * half:2048 * (half + 1)], osb[:])

            # Software-pipeline by one block: the PE stream per block is
            # [proj(j) | attention(j-1)+o_proj(j-1) | norm matmuls(j)], so the
            # ACT/DVE rope + norm chains for block j drain while the PE runs
            # attention for block j-1, and vice versa. The last block's
            # attention runs in two column halves so its o_proj overlaps.
            def staggered_evict(j, pq, pk, pv, attn_j):
                """k/q0/v evicted up front; q1..q3 between attention heads
                so head 0's softmax chain isn't queued behind six casts."""
                _mark(nc, f"proj{j}.evict")
                act0 = attn_j is None
                qraws = [emit_evict(pk, QH, sq_on_act=act0),
                         emit_evict(pq[0], 0, sq_on_act=act0)]
                vt = tmp.tile([D, 512], BF16, tag="vt", name="vt")
                nc.vector.tensor_copy(vt[:], pv[:])
                tails = []
                if attn_j is not None:
                    for h in range(QH):
                        tails.append(emit_attn_head(attn_j, h))
                        if h < QH - 1:
                            qraws.append(emit_evict(pq[h + 1], h + 1))
                    for h, (po_, rd_) in enumerate(tails):
                        finish_attn_head(attn_j, h, po_, rd_)
                else:
                    for h in range(1, QH):
                        qraws.append(emit_evict(pq[h], h, sq_on_act=True))
                return qraws, vt

            for j in range(NJ):
                if j == 0:
                    pq, pk, pv, tabs = emit_proj(0, prefetched=pre0,
                                                 tabs=tabs0)
                else:
                    pq, pk, pv, tabs = emit_proj(j)
                qraws, vt = staggered_evict(j, pq, pk, pv,
                                            j - 1 if j > 0 else None)
                emit_rope(j, qraws, vt, tabs)
                if j == 0:
                    nc.gpsimd.dma_start(
                        wosb[:], wo[:].rearrange("(h p) f -> p h f", p=D))
                if j > 0:
                    emit_oproj(j - 1, range(4 * (j - 1), 4 * j))
            tails = [emit_attn_head(NJ - 1, h) for h in range(QH)]
            for h, (po_, rd_) in enumerate(tails):
                finish_attn_head(NJ - 1, h, po_, rd_)
            emit_oproj(NJ - 1, range(12, 16))
            _mark(nc, None)

    _split_multi_waits(nc)
    return nc


_NC_CACHE = None


def _get_program():
    global _NC_CACHE
    if _NC_CACHE is None:
        _NC_CACHE = build_program()
    return _NC_CACHE


def _rope_tables(cos_g, sin_g, w):
    """Pack [D, 2, S]: [:, 0] = cos_g.T * w[d]; [:, 1] = swS where
    swS[d, s] = sign(pair(d)) * sin_g[s, pair(d)] * w[d], i.e. the rotate
    table with halves pre-swapped so t2[lo] = qraw[hi] * swS[hi] etc."""
    half = D // 2
    cw = np.ascontiguousarray((cos_g * w[None, :]).T)
    swS = np.empty((D, S), np.float32)
    swS[:half, :] = (sin_g[:, half:] * w[:half][None, :]).T
    swS[half:, :] = -(sin_g[:, :half] * w[half:][None, :]).T
    return np.ascontiguousarray(np.stack([cw, swS], axis=1))  # [D, 2, S]


def kernel(x, position_ids, cos, sin, attn_mask, Wq, Wk, Wv, Wo, q_norm_w, k_norm_w):
    x = np.asarray(x, np.float32)
    position_ids = np.asarray(position_ids)
    cos_g = np.asarray(cos, np.float32)[position_ids]   # [S, D]
    sin_g = np.asarray(sin, np.float32)[position_ids]
    Wq = np.asarray(Wq, np.float32)
    Wk = np.asarray(Wk, np.float32)
    Wv = np.asarray(Wv, np.float32)
    Wo = np.asarray(Wo, np.float32)
    qw = np.asarray(q_norm_w, np.float32)
    kw = np.asarray(k_norm_w, np.float32)

    bf = ml_dtypes.bfloat16
    # xt[j, d, hc, e] = x[512j+e, 128hc+d]
    xth = np.ascontiguousarray(
        x.reshape(NJ, 512, NHC, D).transpose(0, 3, 2, 1)).astype(bf)

    import ml_dtypes as _md
    tabq = _rope_tables(cos_g, sin_g, qw).astype(_md.bfloat16)
    tabk = _rope_tables(cos_g, sin_g, kw).astype(_md.bfloat16)
    tri01 = np.triu(np.ones((D, D), np.float32)).astype(bf)

    in_maps = []
    for i in range(NCORES):
        wqkv = np.concatenate([
            Wq[:, QH * D * i:QH * D * (i + 1)],
            Wk[:, D * i:D * (i + 1)],
            Wv[:, D * i:D * (i + 1)],
        ], axis=1)
        # [d, hc, f] partition-major chunks
        wqkv = np.ascontiguousarray(
            wqkv.reshape(NHC, D, (QH + 2) * D).transpose(1, 0, 2)).astype(bf)
        in_maps.append({
            "xt": xth,
            "wqkv": wqkv,
            "wo": np.ascontiguousarray(
                Wo[QH * D * i:QH * D * (i + 1), :]).astype(bf),
            "tabq": tabq, "tabk": tabk,
            "tri01b": tri01,
        })

    nc = _get_program()
    res = run_bass_kernel_spmd(nc, in_maps, list(range(NCORES)))
    acc = np.zeros((S, HID), np.float32)
    for r in res.results:
        acc += np.asarray(r["out"], np.float32)
    return acc


# revision 22
# speedup vs baseline: 1.1818x; 1.1818x over previous
"""GQA causal attention block (sparse_attention) on 8 Trainium2 NeuronCores.

Tensor-parallel over heads: core i computes q-heads 4i..4i+3 and kv-head i
(N_KV == n_cores, so each core owns exactly one kv head), plus the matching
row-slice of the o_proj; the 8 partial o_proj outputs are summed on the host.

Layout choice: everything that feeds the PE keeps the contraction dim on
partitions. Projections produce qT/kT/vT [d, s] directly (stationary = weight
chunk, moving = xT), attention scores are computed transposed [t, s]
(stationary = kT slice, moving = qT), PV consumes v [t, d] (stationary) times
exp-scores [t, s] (moving), and o_proj consumes outT [d, s] as stationary.

Softmax denominators: exp tiles are accumulated elementwise on the Pool
engine (f32), and a single ones-matmul per (head, block) does the final
partition reduction — the PE no longer streams every exp tile twice.
Causal masking on diagonal tiles is a 0/1 upper-triangular multiply on the
exp tile (DVE) instead of a mask-add matmul. Projection/weight DMAs move in
4-chunk groups from partition-major host layouts to cut descriptor-issue
serialization. v tiles are transposed by the DMA XBAR, not the PE.
Reciprocals/rsqrts are exp(-ln(x)) on ACT (one table set: exp/ln/square).
"""

import sys

sys.path.insert(0, "/opt/trn_rl_repo")

import numpy as np
import ml_dtypes

import concourse.bass as bass
import concourse.mybir as mybir
from concourse import tile
from concourse.vector_clock import ScopedClock, VectorClock
from concourse.bass_utils import run_bass_kernel_spmd

F32 = mybir.dt.float32
BF16 = mybir.dt.bfloat16
AF = mybir.ActivationFunctionType
OP = mybir.AluOpType

S = 2048
HID = 4096
N_HEADS = 32
N_KV = 8
D = 128
NCORES = 8
QH = N_HEADS // NCORES          # q heads per core
EPS = 1e-6
SM_SCALE = float(D) ** -0.5
NJ = S // 512                   # 512-wide s blocks
NHC = HID // 128                # 128-deep contraction chunks
NT = S // 128                   # 128-tall t tiles
GC = 4                          # contraction chunks per DMA group
NG = NHC // GC                  # DMA groups per s block


class TileContextFixed(tile.TileContext):
    """TileContext whose tail drain emits one sem-wait per Drain instruction.

    The pinned walrus (CoreV3GenImpl setupSyncWait) rejects instructions that
    carry more than one sync-wait command; stock TileContext attaches the
    whole global clock to a single Drain.
    """

    def _drain_and_barrier(self, tick_clock, wait_clock):
        gc = tick_clock.global_clock
        nprocs = len(gc)
        emitted = False
        for proc in range(nprocs):
            tick = gc[proc]
            if tick <= 0:
                continue
            vec = [0] * nprocs
            vec[proc] = tick
            d = self.nc.sync.drain()
            wait_clock.add_sem_waits(d.ins, ScopedClock({None: VectorClock(vec)}))
            emitted = True
        if not emitted:
            self.nc.sync.drain()

        self.nc.all_engine_barrier()
        assert self.sems is not None
        popped = self.nc._tile_sem_poison_stack.pop()
        assert popped is self._sem_poison
        self.nc.clear_and_free_semaphores(list(self.sems.allocated().values()))
        self.nc.all_engine_barrier()


def _split_multi_waits(nc):
    """Hoist all-but-one sem wait of any instruction onto preceding NOPs.

    The pinned walrus rejects instructions with more than one sync-wait
    command; engine streams execute in order, so a same-engine NOP carrying
    the extra waits right before the instruction is equivalent.
    """
    n = 0
    for f in nc.m.functions:
        for bb in f.blocks:
            rebuilt = []
            changed = False
            for inst in bb.instructions:
                si = inst.sync_info
                if si is not None and len(si.on_wait) > 1:
                    waits = list(si.on_wait)
                    for w in waits[:-1]:
                        n += 1
                        nop = mybir.InstNoOp(
                            name=f"I-waitsplit-{n}",
                            engine=inst.engine,
                            sync_info=mybir.SyncInfo(on_wait=[w], on_update=[]),
                            bass_nofuse=True,
                        )
                        nc.register_instruction(nop)
                        rebuilt.append(nop)
                    inst.sync_info = mybir.SyncInfo(
                        on_wait=[waits[-1]], on_update=list(si.on_update)
                    )
                    changed = True
                rebuilt.append(inst)
            if changed:
                bb.instructions = rebuilt


PHASE_MARKS = []


_SCOPE = [None, None]


def _mark(nc, label):
    if _SCOPE[0] is not None:
        nc.leave_named_scope(_SCOPE[0], _SCOPE[1], False)
        _SCOPE[0] = None
    if label is not None:
        sid, _ = nc.enter_named_scope(label, False)
        _SCOPE[0], _SCOPE[1] = label, sid


def build_program():
    nc = bass.Bass()

    # partition-major host layouts: one DMA covers several contraction chunks
    # xt[j, d, hc, e] = x[512j+e, 128hc+d]
    xt = nc.dram_tensor("xt", [NJ, D, NHC, 512], BF16, kind="ExternalInput")
    # wqkv[d, hc, f] = W[128hc+d, f] with f packed [4*D q | D k | D v]
    wqkv = nc.dram_tensor("wqkv", [D, NHC, (QH + 2) * D], BF16,
                          kind="ExternalInput")
    wo = nc.dram_tensor("wo", [QH * D, HID], BF16, kind="ExternalInput")
    # packed rope tables: [:, 0, :] = cos*w; [:, 1, :] = half-swapped rotate
    # table swS with swS[d] = sign(pair(d))*sin[pair(d)]*w[d], so that
    # rot-half multiplies read both SBUF operands at the same base partition
    tabq = nc.dram_tensor("tabq", [D, 2, S], BF16, kind="ExternalInput")
    tabk = nc.dram_tensor("tabk", [D, 2, S], BF16, kind="ExternalInput")
    tri01b = nc.dram_tensor("tri01b", [D, D], BF16, kind="ExternalInput")
    out = nc.dram_tensor("out", [S, HID], BF16, kind="ExternalOutput")

    with TileContextFixed(nc) as tc:
        with (
            tc.tile_pool(name="const", bufs=1) as constp,
            tc.tile_pool(name="persist", bufs=1) as persist,
            tc.tile_pool(name="wstream", bufs=6) as wstream,
            tc.tile_pool(name="xstream", bufs=6) as xstream,
            tc.tile_pool(name="tmp", bufs=2) as tmp,
            tc.tile_pool(name="tabstream", bufs=4) as tabstream,
            tc.tile_pool(name="expp", bufs=8) as expp,
            tc.tile_pool(name="accp", bufs=3) as accp,
            tc.tile_pool(name="outsb", bufs=2) as outsb,
            tc.tile_pool(name="ps", bufs=8, space="PSUM") as ps,
        ):
            ones = constp.tile([D, D], BF16, tag="ones")
            nc.vector.memset(ones[:], 1.0)

            def issue_group(j, g, split=False):
                xg = xstream.tile([D, GC, 512], BF16, tag="xg", name="xg")
                wg = wstream.tile([D, GC, (QH + 2) * D], BF16, tag="wg",
                                  name="wg")
                if split:
                    hh = GC // 2
                    for s in range(2):
                        cs = slice(GC * g + hh * s, GC * g + hh * (s + 1))
                        ts = slice(hh * s, hh * (s + 1))
                        nc.sync.dma_start(xg[:, ts, :], xt[j, :, cs, :])
                        nc.sync.dma_start(wg[:, ts, :], wqkv[:, cs, :])
                else:
                    nc.sync.dma_start(xg[:], xt[j, :, GC * g:GC * (g + 1), :])
                    nc.sync.dma_start(wg[:], wqkv[:, GC * g:GC * (g + 1), :])
                return xg, wg

            def issue_tabs(j):
                js = slice(512 * j, 512 * (j + 1))
                tabs = []
                for nm, tdram in (("tq", tabq), ("tk", tabk)):
                    tab = tabstream.tile([D, 2, 512], BF16, tag=nm, name=nm)
                    nc.sync.dma_start(tab[:], tdram[:, :, js])
                    tabs.append(tab)
                return tabs

            # prefetch the whole first block's feed before the warmup so the
            # rings fill during the cold preamble
            pre0 = [issue_group(0, g, split=(g < 3)) for g in range(NG)]
            tabs0 = issue_tabs(0)

            epsb = constp.tile([D, 1], F32, tag="epsb")
            nc.vector.memset(epsb[:], EPS)
            tri01 = constp.tile([D, D], BF16, tag="tri01")
            nc.gpsimd.dma_start(tri01[:], tri01b[:])

            wosb = persist.tile([D, QH, HID], BF16, tag="wosb")

            # warm-up matmuls: keep the PE busy during the cold DMA ramp so
            # the HAM clock gate opens before the first projection matmuls
            pwarm = ps.tile([D, 512], F32, tag="ps", name="pwarm")
            for _w in range(32):
                nc.tensor.matmul(pwarm[:, 0:D], ones[:], ones[:],
                                 start=(_w == 0), stop=(_w == 31))

            qhat = [persist.tile([D, S], BF16, tag=f"qhat{h}", name=f"qhat{h}")
                    for h in range(QH)]
            khat = persist.tile([D, S], BF16, tag="khat")
            vsb = persist.tile([D, NT, D], BF16, tag="vsb")
            outt = [persist.tile([D, S], BF16, tag=f"outt{h}", name=f"outt{h}")
                    for h in range(QH)]

            def emit_proj(j, prefetched=None, tabs=None):
                """Projections for s block j + immediate PSUM evictions.

                Returns the evicted raw projections (SBUF) for the rope stage.
                """
                _mark(nc, f"proj{j}")
                pq = [ps.tile([D, 512], F32, tag="ps", name=f"pq{_h}")
                      for _h in range(QH)]
                pk = ps.tile([D, 512], F32, tag="ps", name="pk")
                pv = ps.tile([D, 512], F32, tag="ps", name="pv")
                for g in range(NG):
                    if prefetched is not None:
                        xg, wg = prefetched[g]
                    else:
                        xg, wg = issue_group(j, g)
                    if g == 1 and tabs is None:
                        tabs = issue_tabs(j)
                    for c in range(GC):
                        hc = GC * g + c
                        st = dict(start=(hc == 0), stop=(hc == NHC - 1))
                        for h in range(QH):
                            nc.tensor.matmul(pq[h][:],
                                             wg[:, c, 128 * h:128 * (h + 1)],
                                             xg[:, c, :], **st)
                        nc.tensor.matmul(pk[:], wg[:, c, QH * D:(QH + 1) * D],
                                         xg[:, c, :], **st)
                        nc.tensor.matmul(pv[:], wg[:, c, (QH + 1) * D:],
                                         xg[:, c, :], **st)

                return pq, pk, pv, tabs

            def emit_evict(psrc, h):
                qraw = tmp.tile([D, 512], BF16, tag="qraw", bufs=6,
                                name="qraw")
                nc.vector.tensor_copy(qraw[:], psrc[:])
                sq = tmp.tile([D, 512], BF16, tag="sq", bufs=6, name="sq")
                nc.vector.tensor_tensor(sq[:], qraw[:], qraw[:], OP.mult)
                return (h, qraw, sq)

            def emit_rope(j, qraws, vt, tabs):
                """RMS-norm + rope (k first) + v transpose for s block j."""
                _mark(nc, f"rope{j}")
                js = slice(512 * j, 512 * (j + 1))
                tabqt, tabkt = tabs
                for h, qraw, sq in qraws:
                    if h < QH:
                        dstt, tab = qhat[h], tabqt
                    else:
                        dstt, tab = khat, tabkt
                    pss = ps.tile([D, 512], F32, tag="ps", name="pss")
                    nc.tensor.matmul(pss[:], ones[:], sq[:], start=True,
                                     stop=True)
                    # r = rsqrt(mean + eps) = exp(-0.5 * ln(sumsq/128 + eps))
                    rbc = tmp.tile([D, 512], BF16, tag="rbc", name="rbc")
                    nc.scalar.activation(rbc[:], pss[:], AF.Ln,
                                         bias=epsb[:], scale=1.0 / D)
                    nc.scalar.activation(rbc[:], rbc[:], AF.Exp, bias=0.0,
                                         scale=-0.5)
                    t1 = tmp.tile([D, 512], BF16, tag="t1", name="t1")
                    nc.vector.tensor_tensor(t1[:], qraw[:], tab[:, 0, :],
                                            OP.mult)
                    t2 = tmp.tile([D, 512], BF16, tag="t2", name="t2")
                    nc.vector.tensor_tensor(t2[0:64, :], qraw[64:128, :],
                                            tab[64:128, 1, :], OP.mult)
                    nc.vector.tensor_tensor(t2[64:128, :], qraw[0:64, :],
                                            tab[0:64, 1, :], OP.mult)
                    nc.vector.tensor_tensor(t1[:], t1[:], t2[:], OP.add)
                    nc.vector.tensor_tensor(dstt[:, js], t1[:], rbc[:],
                                            OP.mult)

                for c in range(4):
                    nc.sync.dma_start_transpose(
                        vsb[:, 4 * j + c, :], vt[:, 128 * c:128 * (c + 1)])

            def emit_attn_head(j, h, clo=0, chi=512):
                """Attention for s block j, head h, q cols [clo, chi)."""
                if True:
                    _mark(nc, f"attn{j}.h{h}")
                    po = ps.tile([D, 512], F32, tag="ps", name="po")
                    acc = accp.tile([D, 512], BF16, tag="acc", name="acc")
                    tiles = []
                    for tt in range(4 * j + 4):
                        wlo = 128 * (tt - 4 * j)   # cols < wlo fully masked
                        if wlo >= chi:
                            continue
                        c0 = max(clo, wlo, 0)
                        tiles.append((tt, c0, wlo))
                    pending = []
                    first = True
                    for idx, (tt, c0, wlo) in enumerate(tiles):
                        cs = slice(c0, chi)
                        psc = ps.tile([D, 512], F32, tag="ps", name="psc")
                        nc.tensor.matmul(psc[:, cs],
                                         khat[:, 128 * tt:128 * (tt + 1)],
                                         qhat[h][:, 512 * j + c0:
                                                 512 * j + chi],
                                         start=True, stop=True)
                        ex = expp.tile([D, 512], BF16, tag="ex", name="ex")
                        nc.scalar.activation(ex[:, cs], psc[:, cs], AF.Exp,
                                             bias=0.0, scale=SM_SCALE)
                        if clo <= wlo:   # triangular boundary chunk
                            nc.gpsimd.tensor_tensor(
                                ex[:, wlo:wlo + 128], ex[:, wlo:wlo + 128],
                                tri01[:], OP.mult)
                        # softmax denominator: elementwise-accumulate exp
                        # tiles on DVE in bf16 (all-SBUF 2-byte ops hit the
                        # 4x DVE mode; partition sum comes later from one
                        # ones-matmul)
                        if first:
                            nc.vector.tensor_copy(acc[:, cs], ex[:, cs])
                            first = False
                        else:
                            nc.vector.tensor_tensor(acc[:, cs], acc[:, cs],
                                                    ex[:, cs], OP.add)
                        pending.append((idx, ex, cs))
                        # keep the PE a couple of score tiles ahead of the
                        # exp chain
                        if len(pending) > 2:
                            pidx, pex, pcs = pending.pop(0)
                            nc.tensor.matmul(po[:, pcs],
                                             vsb[:, tiles[pidx][0], :],
                                             pex[:, pcs],
                                             start=(pidx == 0),
                                             stop=(pidx == len(tiles) - 1))
                    for pidx, pex, pcs in pending:
                        nc.tensor.matmul(po[:, pcs], vsb[:, tiles[pidx][0], :],
                                         pex[:, pcs], start=(pidx == 0),
                                         stop=(pidx == len(tiles) - 1))
                    pd = ps.tile([D, 512], F32, tag="ps", name="pd")
                    nc.tensor.matmul(pd[:, clo:chi], ones[:],
                                     acc[:, clo:chi], start=True, stop=True)
                    # only the PSUM-freeing Ln runs inline; the Exp and the
                    # outt normalize are deferred past the head loop so the
                    # ACT queue streams exps uninterrupted during attention
                    rd = tmp.tile([D, 512], F32, tag="rd", bufs=4, name="rd")
                    nc.scalar.activation(rd[:, clo:chi], pd[:, clo:chi],
                                         AF.Ln, bias=0.0, scale=1.0)
                    return po, rd

            def finish_attn_head(j, h, po, rd, clo=0, chi=512):
                rdb = tmp.tile([D, 512], BF16, tag="rdb", bufs=4, name="rdb")
                nc.scalar.activation(rdb[:, clo:chi], rd[:, clo:chi],
                                     AF.Exp, bias=0.0, scale=-1.0)
                nc.vector.tensor_tensor(
                    outt[h][:, 512 * j + clo:512 * j + chi],
                    po[:, clo:chi], rdb[:, clo:chi], OP.mult)

            def emit_oproj(j, stts):
                """o_proj rows for s tiles `stts` (within block j)."""
                _mark(nc, f"oproj{j}")
                for stt in stts:
                    ss = slice(128 * stt, 128 * (stt + 1))
                    for half in range(2):
                        pb = [ps.tile([D, 512], F32, tag="ps", name=f"pb{_b}")
                              for _b in range(4)]
                        for h in range(QH):
                            for b in range(4):
                                col = 2048 * half + 512 * b
                                nc.tensor.matmul(pb[b][:], outt[h][:, ss],
                                                 wosb[:, h, col:col + 512],
                                                 start=(h == 0),
                                                 stop=(h == QH - 1))
                        osb = outsb.tile([D, 2048], BF16, tag="osb",
                                         name="osb")
                        for b in range(4):
                            if b % 2 == 0:
                                nc.scalar.copy(osb[:, 512 * b:512 * (b + 1)],
                                               pb[b][:])
                            else:
                                nc.vector.tensor_copy(
                                    osb[:, 512 * b:512 * (b + 1)], pb[b][:])
                        nc.gpsimd.dma_start(
                            out[ss, 2048 * half:2048 * (half + 1)], osb[:])

            # Software-pipeline by one block: the PE stream per block is
            # [proj(j) | attention(j-1)+o_proj(j-1) | norm matmuls(j)], so the
            # ACT/DVE rope + norm chains for block j drain while the PE runs
            # attention for block j-1, and vice versa. The last block's
            # attention runs in two column halves so its o_proj overlaps.
            def staggered_evict(j, pq, pk, pv, attn_j):
                """k/q0/v evicted up front; q1..q3 between attention heads
                so head 0's softmax chain isn't queued behind six casts."""
                _mark(nc, f"proj{j}.evict")
                qraws = [emit_evict(pk, QH), emit_evict(pq[0], 0)]
                vt = tmp.tile([D, 512], BF16, tag="vt", name="vt")
                nc.vector.tensor_copy(vt[:], pv[:])
                tails = []
                if attn_j is not None:
                    for h in range(QH):
                        tails.append(emit_attn_head(attn_j, h))
                        if h < QH - 1:
                            qraws.append(emit_evict(pq[h + 1], h + 1))
                    for h, (po_, rd_) in enumerate(tails):
                        finish_attn_head(attn_j, h, po_, rd_)
                else:
                    for h in range(1, QH):
                        qraws.append(emit_evict(pq[h], h))
                return qraws, vt

            for j in range(NJ):
                if j == 0:
                    pq, pk, pv, tabs = emit_proj(0, prefetched=pre0,
                                                 tabs=tabs0)
                else:
                    pq, pk, pv, tabs = emit_proj(j)
                qraws, vt = staggered_evict(j, pq, pk, pv,
                                            j - 1 if j > 0 else None)
                emit_rope(j, qraws, vt, tabs)
                if j == 0:
                    nc.gpsimd.dma_start(
                        wosb[:], wo[:].rearrange("(h p) f -> p h f", p=D))
                if j > 0:
                    emit_oproj(j - 1, range(4 * (j - 1), 4 * j))
            tails = [emit_attn_head(NJ - 1, h) for h in range(QH)]
            for h, (po_, rd_) in enumerate(tails):
                finish_attn_head(NJ - 1, h, po_, rd_)
            emit_oproj(NJ - 1, range(12, 16))
            _mark(nc, None)

    _split_multi_waits(nc)
    return nc


_NC_CACHE = None


def _get_program():
    global _NC_CACHE
    if _NC_CACHE is None:
        _NC_CACHE = build_program()
    return _NC_CACHE


def _rope_tables(cos_g, sin_g, w):
    """Pack [D, 2, S]: [:, 0] = cos_g.T * w[d]; [:, 1] = swS where
    swS[d, s] = sign(pair(d)) * sin_g[s, pair(d)] * w[d], i.e. the rotate
    table with halves pre-swapped so t2[lo] = qraw[hi] * swS[hi] etc."""
    half = D // 2
    cw = np.ascontiguousarray((cos_g * w[None, :]).T)
    swS = np.empty((D, S), np.float32)
    swS[:half, :] = (sin_g[:, half:] * w[:half][None, :]).T
    swS[half:, :] = -(sin_g[:, :half] * w[half:][None, :]).T
    return np.ascontiguousarray(np.stack([cw, swS], axis=1))  # [D, 2, S]


def kernel(x, position_ids, cos, sin, attn_mask, Wq, Wk, Wv, Wo, q_norm_w, k_norm_w):
    x = np.asarray(x, np.float32)
    position_ids = np.asarray(position_ids)
    cos_g = np.asarray(cos, np.float32)[position_ids]   # [S, D]
    sin_g = np.asarray(sin, np.float32)[position_ids]
    Wq = np.asarray(Wq, np.float32)
    Wk = np.asarray(Wk, np.float32)
    Wv = np.asarray(Wv, np.float32)
    Wo = np.asarray(Wo, np.float32)
    qw = np.asarray(q_norm_w, np.float32)
    kw = np.asarray(k_norm_w, np.float32)

    bf = ml_dtypes.bfloat16
    # xt[j, d, hc, e] = x[512j+e, 128hc+d]
    xth = np.ascontiguousarray(
        x.reshape(NJ, 512, NHC, D).transpose(0, 3, 2, 1)).astype(bf)

    import ml_dtypes as _md
    tabq = _rope_tables(cos_g, sin_g, qw).astype(_md.bfloat16)
    tabk = _rope_tables(cos_g, sin_g, kw).astype(_md.bfloat16)
    tri01 = np.triu(np.ones((D, D), np.float32)).astype(bf)

    in_maps = []
    for i in range(NCORES):
        wqkv = np.concatenate([
            Wq[:, QH * D * i:QH * D * (i + 1)],
            Wk[:, D * i:D * (i + 1)],
            Wv[:, D * i:D * (i + 1)],
        ], axis=1)
        # [d, hc, f] partition-major chunks
        wqkv = np.ascontiguousarray(
            wqkv.reshape(NHC, D, (QH + 2) * D).transpose(1, 0, 2)).astype(bf)
        in_maps.append({
            "xt": xth,
            "wqkv": wqkv,
            "wo": np.ascontiguousarray(
                Wo[QH * D * i:QH * D * (i + 1), :]).astype(bf),
            "tabq": tabq, "tabk": tabk,
            "tri01b": tri01,
        })

    nc = _get_program()
    res = run_bass_kernel_spmd(nc, in_maps, list(range(NCORES)))
    acc = np.zeros((S, HID), np.float32)
    for r in res.results:
        acc += np.asarray(r["out"], np.float32)
    return acc
